# revision 16
# baseline (speedup 1.0000x reference)
"""Causal multi-head self-attention with RoPE on 8 Trainium2 NeuronCores.

Problem shapes (hardcoded): x [2, 2048, 1024], wq/wk/wv/wo [1024, 1024],
16 heads, head dim 64, rope theta 1000.0.

Sharding: tensor-parallel over heads — each of the 8 cores owns 2 heads
(128 of the 1024 hidden dims). wq/wk/wv are column-sharded (rows of the
[out, in] weights), wo is row-sharded; the all-reduce of the 8 partial
outputs is done on the host during the gather/unshard step.

Per-core layout choices:
  - x is pre-transposed on the host to xT [B, dc, qt, 128d, 512t] (bf16)
    so projection matmuls consume it directly as the moving operand.
  - wq/wk rows are permuted per head so RoPE (even, odd) pairs become
    (row r, row r+32) within each 64-row head block: RoPE is then
    full-width elementwise ops plus one 32-row partition swap done by DMA.
  - Attention computes S^T tiles [128 keys, 512 queries] so that exp(S^T)
    feeds the P@V matmul directly as the moving operand (no transposes).
    Softmax needs no max-subtraction (scores are O(1) here), and the
    denominator comes free by augmenting V with a ones column.
  - All matmul inputs are bf16 (1 cycle/row on the PE); accumulation is
    fp32 in PSUM.
"""

import sys

sys.path.insert(0, "/opt/trn_rl_repo")

import ml_dtypes
import numpy as np

import concourse.bacc as bacc
import concourse.tile as tile
from concourse import mybir

F32 = mybir.dt.float32
BF16 = mybir.dt.bfloat16

B = 2
T = 2048
D = 1024
H = 16
DK = 64
NCORES = 8
HPC = H // NCORES      # heads per core = 2
E = HPC * DK           # local out dims per core = 128
DC = D // 128          # 8 chunks of the contraction dim
QT = T // 512          # 4 query tiles of 512
TT = T // 128          # 16 key/value tiles of 128
ROPE_THETA = 1000.0


def build_nc():
    nc = bacc.Bacc("TRN2", target_bir_lowering=False, debug=False,
                   num_devices=NCORES)

    xT = nc.dram_tensor("xT", [B, DC, QT, 128, 512], BF16, kind="ExternalInput")
    wqT = nc.dram_tensor("wqT", [DC, 128, E], BF16, kind="ExternalInput")
    wkT = nc.dram_tensor("wkT", [DC, 128, E], BF16, kind="ExternalInput")
    wvT = nc.dram_tensor("wvT", [DC, 128, E], BF16, kind="ExternalInput")
    woT = nc.dram_tensor("woT", [DC, E, 128], BF16, kind="ExternalInput")
    ctab = nc.dram_tensor("ctab", [128, T], F32, kind="ExternalInput")
    stab = nc.dram_tensor("stab", [128, T], F32, kind="ExternalInput")
    tri = nc.dram_tensor("tri", [128, 128], BF16, kind="ExternalInput")
    eye = nc.dram_tensor("eye", [128, 128], BF16, kind="ExternalInput")
    outT = nc.dram_tensor("outT", [B, QT, DC, 128, 512], F32, kind="ExternalOutput")

    from contextlib import ExitStack

    with tile.TileContext(nc) as tc, ExitStack() as est:
        if True:
            pool = lambda name, bufs, **kw: est.enter_context(
                tc.tile_pool(name=name, bufs=bufs, **kw))
            constp = pool("const", 1)
            xtp = pool("xt", 10)
            rotp = pool("rot", 4)
            vtp = pool("vt", 2)
            stagep = pool("stage", 2)
            swpp = pool("swp", 2)
            m1p = pool("m1", 2)
            m2p = pool("m2", 2)
            ptp = pool("pt", 3)
            catp = pool("cat", 2)
            osbp = pool("osb", 3)
            recp = pool("rec", 2)
            psA = pool("psA", 2, space="PSUM")   # proj (qk + v) + recip bcast
            psS = pool("psS", 2, space="PSUM")   # S^T tiles
            psH = pool("psH", 2, space="PSUM")   # head accum
            psO = pool("psO", 2, space="PSUM")   # wo out
            # ---- constants ----
            wq_sb = constp.tile([128, DC, E], BF16, tag="wq")
            wk_sb = constp.tile([128, DC, E], BF16, tag="wk")
            wv_sb = constp.tile([128, DC, E], BF16, tag="wv")
            wo_sb = constp.tile([128, DC, 128], BF16, tag="wo")
            nc.sync.dma_start(wq_sb[:], wqT[:].transpose([1, 0, 2]))
            nc.sync.dma_start(wk_sb[:], wkT[:].transpose([1, 0, 2]))
            nc.sync.dma_start(wv_sb[:], wvT[:].transpose([1, 0, 2]))
            nc.sync.dma_start(wo_sb[:], woT[:].transpose([1, 0, 2]))
            ct_sb = constp.tile([128, T], F32, tag="ct")
            st_sb = constp.tile([128, T], F32, tag="st")
            tri_sb = constp.tile([128, 128], BF16, tag="tri")
            nc.sync.dma_start(ct_sb[:], ctab[:])
            nc.sync.dma_start(st_sb[:], stab[:])
            nc.sync.dma_start(tri_sb[:], tri[:])
            eye_sb = constp.tile([128, 128], BF16, tag="eye")
            nc.sync.dma_start(eye_sb[:], eye[:])
            ones_sb = constp.tile([1, DK], BF16, tag="ones")
            nc.vector.memset(ones_sb[:], 1.0)

            for b in range(B):
                # ---- load xT for this batch ----
                xts = []
                for dc in range(DC):
                    xt = xtp.tile([128, T], BF16, tag="xt")
                    for qt in range(QT):
                        nc.sync.dma_start(xt[:, qt * 512:(qt + 1) * 512],
                                          xT[b, dc, qt])
                    xts.append(xt)

                # ---- Q/K projections + RoPE ----
                rots = {}
                for name, w_sb in (("q", wq_sb), ("k", wk_sb)):
                    rot = rotp.tile([128, T], BF16, tag="rot")
                    rots[name] = rot
                    for qt in range(QT):
                        ps = psA.tile([128, 512], F32, tag="psA")
                        for dc in range(DC):
                            nc.tensor.matmul(
                                ps[:], w_sb[:, dc, :],
                                xts[dc][:, qt * 512:(qt + 1) * 512],
                                start=(dc == 0), stop=(dc == DC - 1))
                        # RoPE: rot = ps * C + swap32(ps) * S
                        stage = stagep.tile([128, 512], F32, tag="stage")
                        nc.vector.tensor_copy(stage[:], ps[:])
                        swp = swpp.tile([128, 512], F32, tag="swp")
                        for half in range(4):
                            lo, hi = half * 32, half * 32 + 32
                            src = (half ^ 1)
                            slo, shi = src * 32, src * 32 + 32
                            nc.sync.dma_start(swp[lo:hi, :], stage[slo:shi, :])
                        cs = ct_sb[:, qt * 512:(qt + 1) * 512]
                        ss = st_sb[:, qt * 512:(qt + 1) * 512]
                        m1 = m1p.tile([128, 512], F32, tag="m1")
                        nc.vector.tensor_mul(m1[:], ps[:], cs)
                        m2 = m2p.tile([128, 512], F32, tag="m2")
                        nc.gpsimd.tensor_mul(m2[:], swp[:], ss)
                        nc.vector.tensor_add(
                            rot[:, qt * 512:(qt + 1) * 512], m1[:], m2[:])

                # ---- V projection (V^T matmuls, PE-transposed to natural,
                #      ones-augmented) ----
                vt = vtp.tile([128, TT, 2, 65], BF16, tag="vt")
                nc.vector.memset(vt[:, :, :, 64:65], 1.0)
                for t4 in range(QT):
                    psv = psA.tile([128, 512], F32, tag="psA")
                    for dc in range(DC):
                        nc.tensor.matmul(
                            psv[:], wv_sb[:, dc, :],
                            xts[dc][:, t4 * 512:(t4 + 1) * 512],
                            start=(dc == 0), stop=(dc == DC - 1))
                    vts = swpp.tile([128, 512], BF16, tag="vts")
                    nc.vector.tensor_copy(vts[:], psv[:])
                    for j in range(4):
                        pst = psO.tile([128, 128], BF16, tag="psO")
                        nc.tensor.transpose(pst[:], vts[:, j * 128:(j + 1) * 128],
                                            eye_sb[:])
                        nc.scalar.copy(
                            vt[:, t4 * 4 + j, :, 0:64],
                            pst[:].rearrange("p (j k) -> p j k", j=2))

                # ---- attention + wo, per query tile ----
                qrot, krot = rots["q"], rots["k"]
                for qt in range(QT):
                    cat = catp.tile([128, 512], BF16, tag="cat")
                    for h in range(HPC):
                        ph = psH.tile([65, 512], F32, tag="psH")
                        nkb = 4 * qt + 4
                        for kb in range(nkb):
                            j0 = max(0, kb - 4 * qt)
                            c0 = j0 * 128
                            ks = krot[h * 64:(h + 1) * 64,
                                      kb * 128:(kb + 1) * 128]
                            qs = qrot[h * 64:(h + 1) * 64,
                                      qt * 512 + c0:(qt + 1) * 512]
                            pss = psS.tile([128, 512], F32, tag="psS")
                            nc.tensor.matmul(pss[:, c0:512], ks, qs,
                                             start=True, stop=True)
                            pt = ptp.tile([128, 512], BF16, tag="pt")
                            nc.scalar.activation(
                                pt[:, c0:512], pss[:, c0:512],
                                mybir.ActivationFunctionType.Exp,
                                scale=float(1.0 / np.sqrt(DK)))
                            if kb - 4 * qt >= 0:
                                nc.vector.tensor_mul(
                                    pt[:, c0:c0 + 128], pt[:, c0:c0 + 128],
                                    tri_sb[:])
                            nc.tensor.matmul(
                                ph[:, c0:512], vt[:, kb, h, 0:65],
                                pt[:, c0:512],
                                start=(kb == 0), stop=(kb == nkb - 1))
                        # normalize: cat[h] = ph[0:64] * bcast(1 / ph[64])
                        # (reciprocal_approx_fast mis-reads PSUM on HW;
                        # stage the denominator row through SBUF first)
                        lrow = recp.tile([1, 512], F32, tag="lrow")
                        nc.vector.tensor_copy(lrow[:], ph[64:65, :])
                        rec_f = recp.tile([1, 512], F32, tag="recf")
                        nc.vector.reciprocal_approx_fast(rec_f[:], lrow[:])
                        rec = recp.tile([1, 512], BF16, tag="rec")
                        nc.vector.tensor_copy(rec[:], rec_f[:])
                        pb = psO.tile([64, 512], F32, tag="psO")
                        nc.tensor.matmul(pb[:], ones_sb[:], rec[:],
                                         start=True, stop=True)
                        pb_sb = recp.tile([64, 512], BF16, tag="pbsb")
                        nc.vector.tensor_copy(pb_sb[:], pb[:])
                        nc.vector.tensor_mul(cat[h * 64:(h + 1) * 64, :],
                                             ph[0:64, :], pb_sb[:])
                    # wo projection: outT[b, qt, ec] = woT[ec].T @ cat
                    for ec in range(DC):
                        po = psO.tile([128, 512], F32, tag="psO")
                        nc.tensor.matmul(po[:], wo_sb[:, ec, :], cat[:],
                                         start=True, stop=True)
                        osb = osbp.tile([128, 512], F32, tag="osb")
                        if ec % 2 == 0:
                            nc.scalar.copy(osb[:], po[:])
                        else:
                            nc.vector.tensor_copy(osb[:], po[:])
                        nc.sync.dma_start(outT[b, qt, ec], osb[:])
    nc.compile()
    return nc


_NC_CACHE = None


def _get_nc():
    global _NC_CACHE
    if _NC_CACHE is None:
        _NC_CACHE = build_nc()
    return _NC_CACHE


def make_inputs(x, wq, wk, wv, wo, core):
    """Per-core input prep (numpy). core in [0, 8)."""
    bf16 = ml_dtypes.bfloat16
    # xT [B, dc, qt, 128, 512]; identical for every core
    xt = np.ascontiguousarray(
        x.transpose(0, 2, 1).reshape(B, DC, 128, QT, 512).transpose(0, 1, 3, 2, 4)
    ).astype(bf16)

    # per-head even/odd de-interleave permutation for q/k rows
    perm64 = np.concatenate([np.arange(0, 64, 2), np.arange(1, 64, 2)])
    rows = core * 128 + (np.arange(128) // 64) * 64 + perm64[np.arange(128) % 64]
    rows_plain = core * 128 + np.arange(128)

    def wT_blocks(w, rws):
        # [dc, 128d, 128e] with [dc, d, e] = w[rws[e], dc*128 + d]
        return np.ascontiguousarray(
            w[rws, :].T.reshape(DC, 128, E))

    wqT = wT_blocks(wq, rows).astype(bf16)
    wkT = wT_blocks(wk, rows).astype(bf16)
    wvT = wT_blocks(wv, rows_plain).astype(bf16)
    # woT [ec, d_local, e_out] = wo[ec*128 + e, core*128 + d]
    woT = np.ascontiguousarray(
        wo[:, core * 128:(core + 1) * 128].reshape(DC, 128, 128).transpose(0, 2, 1)
    ).astype(bf16)

    inv = ROPE_THETA ** (-2.0 * np.arange(DK // 2) / DK)
    ang = np.arange(T)[None, :] * inv[:, None]          # [32, T]
    cos32 = np.cos(ang).astype(np.float32)
    sin32 = np.sin(ang).astype(np.float32)
    ctab = np.tile(cos32, (4, 1))
    stab = np.tile(np.concatenate([-sin32, sin32], axis=0), (2, 1))
    tri = (np.arange(128)[:, None] <= np.arange(128)[None, :]).astype(bf16)
    eye = np.eye(128).astype(bf16)

    return {
        "xT": xt, "wqT": wqT, "wkT": wkT, "wvT": wvT, "woT": woT,
        "ctab": ctab, "stab": stab, "tri": tri, "eye": eye,
    }


def gather_output(results):
    """Sum per-core partials and restore [B, T, D] layout."""
    acc = None
    for res in results:
        o = np.asarray(res["outT"], dtype=np.float32)
        acc = o if acc is None else acc + o
    # outT[b, qt, ec, e, q] -> out[b, qt*512+q, ec*128+e]
    return np.ascontiguousarray(
        acc.transpose(0, 1, 4, 2, 3).reshape(B, T, D))


def kernel(x, wq, wk, wv, wo, trace=False, **run_kwargs):
    from concourse.bass_utils import run_bass_kernel_spmd

    x = np.asarray(x, dtype=np.float32)
    wq = np.asarray(wq, dtype=np.float32)
    wk = np.asarray(wk, dtype=np.float32)
    wv = np.asarray(wv, dtype=np.float32)
    wo = np.asarray(wo, dtype=np.float32)

    nc = _get_nc()
    in_maps = [make_inputs(x, wq, wk, wv, wo, c) for c in range(NCORES)]
    res = run_bass_kernel_spmd(nc, in_maps, core_ids=list(range(NCORES)),
                               trace=trace, **run_kwargs)
    out = gather_output(res.results)
    kernel.last_results = res
    return out
